# revision 13
# baseline (speedup 1.0000x reference)
"""Trainium2 Bass kernel for nn_MultiHeadedAttention_52037823758722.

Reference computation (per batch b of B=32, L=1024, D=512, H=8 heads, W=5):
  q = local_mix(query, W0, b0, 5)   # sliding-window softmax mixing of y=x@W0+b0
  k = local_mix(key,   W0, b0, 5)
  v = value @ W0 + b0
  full softmax attention per head (dk=64), then out = attn @ Wout + bout.

Sharding: data-parallel over batch across 8 NeuronCores (4 batches/core),
weights replicated, no collectives.

Layout strategy per core (everything bf16 for matmuls, fp32 accumulation):
  - inputs x are cast to bf16 and DMA-xbar-transposed to x^T [D, L]
  - linear outputs y^T = W0^T @ x^T kept feature-major [D, L] (ACT adds b0)
  - y natural [L, D] recovered via DMA transpose for the window weighted sum
  - band scores S = Y_blk @ Y_ext^T computed per 128-query block with a
    5-wide diagonal additive mask; exp on ACT with fused row-sum (accum_out)
  - window weighted sum emits qm^T / km^T directly (feature-major)
  - attention scores computed TRANSPOSED (S^T[key, query]) so the exp'd
    P^T feeds P@V directly with keys on the contraction partitions;
    a ones-column appended to V yields the softmax normalizer for free
  - O^T accumulates head-major into X_att^T, which is exactly the lhsT
    layout the output projection wants; final Z lands natural and DMAs out.
"""

import math

import numpy as np

import ml_dtypes

import concourse.bass as bass
import concourse.mybir as mybir
import concourse.tile as tile

F32 = mybir.dt.float32
BF16 = mybir.dt.bfloat16

B, L, D, H, W = 32, 1024, 512, 8, 5
DK = D // H  # 64
NCORES = 8
BL = B // NCORES  # 4 batches per core
P = 128
FC = D // P  # 4 feature chunks of 128
RC = L // P  # 8 row chunks of 128 per batch
QG = 512  # query group width for attention
NQG = L // QG  # 2
KC = L // P  # 8 key chunks
EXT = 4 + L + 124  # 1152: 4 phantom b0 rows + L rows + zero tail
NEG = -30000.0
BAND_SCALE = 1.0 / math.sqrt(D)
ATT_SCALE = 1.0 / math.sqrt(DK)


def build_nc():
    nc = bass.Bass(target_bir_lowering=False, trn_type="TRN2")

    x_dram = {
        t: nc.dram_tensor(t, [BL, L, D], F32, kind="ExternalInput")
        for t in ("query", "key", "value")
    }
    w0_dram = nc.dram_tensor("W0", [D, D], F32, kind="ExternalInput")
    b0_dram = nc.dram_tensor("b0", [D], F32, kind="ExternalInput")
    wout_dram = nc.dram_tensor("Wout", [D, D], F32, kind="ExternalInput")
    bout_dram = nc.dram_tensor("bout", [D], F32, kind="ExternalInput")
    out_dram = nc.dram_tensor("out", [BL, L, D], F32, kind="ExternalOutput")

    with tile.TileContext(nc) as tc:
        build_body(tc, x_dram, w0_dram, b0_dram, wout_dram, bout_dram, out_dram)
    _patch_multiwait_dmas(nc)
    return nc


def _patch_multiwait_dmas(nc):
    """walrus's DMA pseudo-instructions encode at most one sync-wait command
    (plus the completion update). Tile can emit DMAs waiting on several
    producer procs. Move the extra waits onto a NoOp right before the DMA on
    the issuing engine's stream: the sequencer blocks on the NoOp's waits,
    then enqueues the DMA — identical synchronization, legal encoding."""
    for f in nc.m.functions:
        for blk in f.blocks:
            insts = blk.instructions
            out = []
            changed = False
            for inst in insts:
                tname = type(inst).__name__
                fixable = "NoOp" not in tname and "Branch" not in tname
                si = inst.sync_info
                if fixable and si is not None and len(si.on_wait) > 1:
                    changed = True
                    waits = list(si.on_wait)
                    for i in range(0, len(waits), 1):
                        nop = mybir.InstNoOp(
                            name=nc.get_next_instruction_name(),
                            engine=inst.engine,
                            ins=[],
                            outs=[],
                            sync_info=mybir.SyncInfo(
                                on_wait=waits[i:i + 1], on_update=[]
                            ),
                            bass_nofuse=True,
                        )
                        nc.register_instruction(nop)
                        out.append(nop)
                    inst.sync_info = mybir.SyncInfo(
                        on_wait=[], on_update=list(si.on_update)
                    )
                out.append(inst)
            if changed:
                blk.instructions = out


def build_body(tc, x_dram, w0_dram, b0_dram, wout_dram, bout_dram, out_dram):
    nc = tc.nc
    from contextlib import ExitStack

    ctx = ExitStack()
    singles = ctx.enter_context(tc.tile_pool(name="singles", bufs=1))
    stage = ctx.enter_context(tc.tile_pool(name="stage", bufs=2))
    xTp = ctx.enter_context(tc.tile_pool(name="xT", bufs=1))
    bigp = ctx.enter_context(tc.tile_pool(name="big", bufs=1))
    smalls = ctx.enter_context(tc.tile_pool(name="smalls", bufs=3))
    patt_p = ctx.enter_context(tc.tile_pool(name="patt", bufs=2))
    outp = ctx.enter_context(tc.tile_pool(name="outp", bufs=1))
    dramp = ctx.enter_context(tc.tile_pool(name="dramp", bufs=3, space="DRAM"))
    psA = ctx.enter_context(tc.tile_pool(name="psA", bufs=2, space="PSUM"))
    psS = ctx.enter_context(tc.tile_pool(name="psS", bufs=2, space="PSUM"))
    psO = ctx.enter_context(tc.tile_pool(name="psO", bufs=2, space="PSUM"))

    # ---- constants -------------------------------------------------------
    # weights, striped [p, kc, f] with contraction dim on partitions
    w0_bf = singles.tile([P, FC, D], BF16)
    wout_bf = singles.tile([P, FC, D], BF16)
    for wdram, wbf in ((w0_dram, w0_bf), (wout_dram, wout_bf)):
        wf32 = stage.tile([P, RC, D], F32, tag="xf")
        nc.sync.dma_start(
            out=wf32[:, :FC, :], in_=wdram[:].rearrange("(kc p) f -> p kc f", p=P)
        )
        nc.vector.tensor_copy(wbf, wf32[:, :FC, :])

    # b0 striped per-partition for feature-major bias add: b0_part[p, fc]
    b0_part = singles.tile([P, FC], F32)
    nc.sync.dma_start(out=b0_part, in_=b0_dram[:].rearrange("(fc p) -> p fc", p=P))
    # b0 / bout replicated across partitions for natural-layout bias adds
    b0_row = singles.tile([P, D], F32)
    nc.sync.dma_start(out=b0_row, in_=b0_dram[:][None, :].partition_broadcast(P))
    bout_row = singles.tile([P, D], F32)
    nc.sync.dma_start(out=bout_row, in_=bout_dram[:][None, :].partition_broadcast(P))
    # b0 in bf16 for the phantom window rows
    b0_part_bf = singles.tile([P, FC], BF16)
    nc.vector.tensor_copy(b0_part_bf, b0_part)

    identity = singles.tile([P, P], BF16)
    ident_dram = nc.inline_tensor(np.eye(P, dtype=ml_dtypes.bfloat16), name="ident")
    nc.sync.dma_start(out=identity, in_=ident_dram[:])

    # additive band mask [128 queries, 256 ext keys]: 0 where 0<=c-p<=4 else NEG
    mask_np = np.full((P, 256), NEG, dtype=np.float32)
    for p_ in range(P):
        mask_np[p_, p_: p_ + W] = 0.0
    mask = singles.tile([P, 256], F32)
    mask_dram = nc.inline_tensor(mask_np, name="bandmask")
    nc.sync.dma_start(out=mask, in_=mask_dram[:])

    for b in range(BL):
        build_batch(
            tc, b, x_dram, out_dram,
            w0_bf, wout_bf, b0_part, b0_part_bf, b0_row, bout_row, identity, mask,
            stage, xTp, bigp, smalls, patt_p, outp, psA, psS, psO, dramp,
        )
    ctx.close()


def build_batch(
    tc, b, x_dram, out_dram,
    w0_bf, wout_bf, b0_part, b0_part_bf, b0_row, bout_row, identity, mask,
    stage, xTp, bigp, smalls, patt_p, outp, psA, psS, psO, dramp,
):
    nc = tc.nc

    # ---- stage A: load inputs, cast to bf16, transpose to [D, L] ---------
    xT = {}
    for t in ("query", "key", "value"):
        xf = stage.tile([P, RC, D], F32, tag="xf")
        nc.sync.dma_start(
            out=xf, in_=x_dram[t][b].rearrange("(rc p) d -> p rc d", p=P)
        )
        xb = stage.tile([P, RC, D], BF16, tag="xb")
        nc.vector.tensor_copy(xb, xf)
        xTt = xTp.tile([P, FC, L], BF16, tag=f"xT_{t}")
        for dc in range(FC):
            for rc in range(RC):
                nc.sync.dma_start_transpose(
                    out=xTt[:, dc, rc * P:(rc + 1) * P],
                    in_=xb[:, rc, dc * P:(dc + 1) * P],
                )
        xT[t] = xTt

    # ---- stage B: linears -------------------------------------------------
    # y^T (feature-major, with 4 phantom b0 cols and zero tail) for q and k
    yT_ext = {}
    y_nat = {}
    for t in ("query", "key"):
        yt = bigp.tile([P, FC, EXT], BF16, tag=f"yT_{t}")
        for fc in range(FC):
            nc.vector.tensor_copy(
                yt[:, fc, 0:4], b0_part_bf[:, fc:fc + 1].to_broadcast([P, 4])
            )
        nc.vector.memset(yt[:, :, 4 + L:], 0.0)
        for fc in range(FC):
            for rg in range(2):
                ps = psA.tile([P, 512], F32, tag="lin")
                for kc in range(FC):
                    nc.tensor.matmul(
                        ps,
                        lhsT=w0_bf[:, kc, fc * P:(fc + 1) * P],
                        rhs=xT[t][:, kc, rg * 512:(rg + 1) * 512],
                        start=(kc == 0),
                        stop=(kc == FC - 1),
                    )
                nc.scalar.add(
                    out=yt[:, fc, 4 + rg * 512: 4 + (rg + 1) * 512],
                    in_=ps,
                    add=b0_part[:, fc:fc + 1],
                )
        yT_ext[t] = yt
        # natural layout [ext_row, D] via DMA transpose (9 chunks of 128 rows)
        yn = bigp.tile([P, EXT // P, D], BF16, tag=f"ynat_{t}")
        for c in range(EXT // P):
            for dc in range(FC):
                nc.sync.dma_start_transpose(
                    out=yn[:, c, dc * P:(dc + 1) * P],
                    in_=yt[:, dc, c * P:(c + 1) * P],
                )
        y_nat[t] = yn

    # v natural layout with ones column per head: v_ext[p, rc, h, 0:64]=v, [..,64]=1
    v_ext = bigp.tile([P, RC, H, DK + 1], BF16, tag="v_ext")
    nc.vector.memset(v_ext[:, :, :, DK:DK + 1], 1.0)
    for rc in range(RC):
        ps = psA.tile([P, 512], F32, tag="lin")
        for kc in range(FC):
            nc.tensor.matmul(
                ps,
                lhsT=xT["value"][:, kc, rc * P:(rc + 1) * P],
                rhs=w0_bf[:, kc, :],
                start=(kc == 0),
                stop=(kc == FC - 1),
            )
        nc.vector.tensor_tensor(
            out=v_ext[:, rc, :, 0:DK],
            in0=ps[:, :].rearrange("p (h d) -> p h d", h=H),
            in1=b0_row[:, :].rearrange("p (h d) -> p h d", h=H),
            op=mybir.AluOpType.add,
        )

    # ---- stage C: local window mix for q and k ---------------------------
    mT = {}
    for t in ("query", "key"):
        yt = yT_ext[t]
        yn = y_nat[t]
        mt = bigp.tile([P, FC, L], BF16, tag=f"mT_{t}")
        for blk in range(RC):
            R = blk * P
            ps_s = psA.tile([P, 512], F32, tag="lin", name="ps_s")[:, :256]
            for fc in range(FC):
                nc.tensor.matmul(
                    ps_s,
                    lhsT=yt[:, fc, 4 + R: 4 + R + P],
                    rhs=yt[:, fc, R: R + 256],
                    start=(fc == 0),
                    stop=(fc == FC - 1),
                )
            nc.vector.tensor_tensor(
                out=ps_s, in0=ps_s, in1=mask, op=mybir.AluOpType.add
            )
            p_band = smalls.tile([P, 256], BF16, tag="p_band")
            ssum = smalls.tile([P, 1], F32, tag="ssum")
            nc.scalar.activation(
                out=p_band, in_=ps_s,
                func=mybir.ActivationFunctionType.Exp,
                scale=BAND_SCALE,
                accum_out=ssum,
            )
            rband = smalls.tile([P, 1], F32, tag="rband")
            nc.vector.reciprocal(rband, ssum)
            nc.vector.tensor_scalar_mul(p_band, p_band, rband)
            pT = smalls.tile([P, 2, P], BF16, tag="pT")
            for half in range(2):
                ps_t = psA.tile([P, 1024], BF16, tag="lin", name="ps_t")[:, :P]
                nc.tensor.transpose(
                    ps_t, p_band[:, half * P:(half + 1) * P], identity
                )
                nc.vector.tensor_copy(pT[:, half, :], ps_t)
            ps_w = psA.tile([P, 512], F32, tag="lin")
            for fc in range(FC):
                for half in range(2):
                    nc.tensor.matmul(
                        ps_w[:, fc * P:(fc + 1) * P],
                        lhsT=yn[:, blk + half, fc * P:(fc + 1) * P],
                        rhs=pT[:, half, :],
                        start=(half == 0),
                        stop=(half == 1),
                    )
            nc.vector.tensor_copy(
                mt[:, :, R:R + P],
                ps_w[:, :].rearrange("p (fc q) -> p fc q", fc=FC),
            )
        mT[t] = mt
    qmT = mT["query"]
    kmT = mT["key"]

    # ---- stage D: full attention per head --------------------------------
    xattT = bigp.tile([P, FC, L], BF16, tag="xattT")
    for h in range(H):
        po = (h % 2) * DK
        fch = h // 2
        for qg in range(NQG):
            pT_att = patt_p.tile([P, KC, QG], BF16, tag="pT_att")
            for kcp in range(KC // 2):
                ps_s2 = psS.tile([P, 1024], F32, tag="att_s")
                for j in range(2):
                    kc = kcp * 2 + j
                    for half in range(2):
                        nc.tensor.matmul(
                            ps_s2[half * DK:(half + 1) * DK, j * QG:(j + 1) * QG],
                            lhsT=kmT[po:po + DK, fch,
                                     kc * P + half * DK: kc * P + (half + 1) * DK],
                            rhs=qmT[po:po + DK, fch, qg * QG:(qg + 1) * QG],
                            start=True,
                            stop=True,
                            tile_position=(po, half * DK),
                        )
                nc.scalar.activation(
                    out=pT_att[:, kcp * 2:(kcp + 1) * 2, :],
                    in_=ps_s2,
                    func=mybir.ActivationFunctionType.Exp,
                    scale=ATT_SCALE,
                )
            ps_o = psO.tile([P, QG], F32, tag="att_o")
            for kc in range(KC):
                nc.tensor.matmul(
                    ps_o[0:DK + 1, :],
                    lhsT=v_ext[:, kc, h, :],
                    rhs=pT_att[:, kc, :],
                    start=(kc == 0),
                    stop=(kc == KC - 1),
                )
            r1 = smalls.tile([1, QG], F32, tag="r1")
            nc.vector.reciprocal(r1, ps_o[DK:DK + 1, :])
            rscr = dramp.tile([1, QG], F32, tag="rscr")
            nc.sync.dma_start(out=rscr, in_=r1)
            rbc = smalls.tile([DK, QG], F32, tag="rbc")
            nc.gpsimd.dma_start(out=rbc, in_=rscr.partition_broadcast(DK))
            nc.vector.tensor_tensor(
                out=xattT[po:po + DK, fch, qg * QG:(qg + 1) * QG],
                in0=ps_o[0:DK, :],
                in1=rbc,
                op=mybir.AluOpType.mult,
            )

    # ---- stage E: output projection --------------------------------------
    outb = outp.tile([P, RC, D], F32, tag="outb")
    for rc in range(RC):
        ps = psA.tile([P, 512], F32, tag="lin")
        for fc in range(FC):
            nc.tensor.matmul(
                ps,
                lhsT=xattT[:, fc, rc * P:(rc + 1) * P],
                rhs=wout_bf[:, fc, :],
                start=(fc == 0),
                stop=(fc == FC - 1),
            )
        nc.vector.tensor_tensor(
            out=outb[:, rc, :], in0=ps, in1=bout_row, op=mybir.AluOpType.add
        )
    nc.sync.dma_start(
        out=out_dram[b].rearrange("(rc p) d -> p rc d", p=P), in_=outb
    )


_NC_CACHE = None


def get_nc():
    global _NC_CACHE
    if _NC_CACHE is None:
        _NC_CACHE = build_nc()
    return _NC_CACHE


def kernel(**inputs) -> np.ndarray:
    from concourse.bass_utils import run_bass_kernel_spmd

    nc = get_nc()
    core_ids = list(range(NCORES))
    in_maps = []
    for c in range(NCORES):
        sl = slice(c * BL, (c + 1) * BL)
        in_maps.append(
            {
                "query": np.ascontiguousarray(inputs["query"][sl], dtype=np.float32),
                "key": np.ascontiguousarray(inputs["key"][sl], dtype=np.float32),
                "value": np.ascontiguousarray(inputs["value"][sl], dtype=np.float32),
                "W0": np.ascontiguousarray(inputs["W0"], dtype=np.float32),
                "b0": np.ascontiguousarray(inputs["b0"], dtype=np.float32),
                "Wout": np.ascontiguousarray(inputs["Wout"], dtype=np.float32),
                "bout": np.ascontiguousarray(inputs["bout"], dtype=np.float32),
            }
        )
    res = run_bass_kernel_spmd(nc, in_maps, core_ids)
    return np.concatenate([r["out"] for r in res.results], axis=0)
